# revision 31
# baseline (speedup 1.0000x reference)
"""Trainium2 Bass kernel for a dense transformer block (RoPE attention + SwiGLU).

Sharding (8 NeuronCores, Megatron-style):
  - QKV + attention: tensor-parallel over heads (2 heads/core, both batches).
  - Two half-AllToAlls (one per batch) reshard attention output from
    head-sharded to token-sharded; the batch-0 collective overlaps with
    batch-1 attention compute (token slices are interleaved per batch so
    both collectives are symmetric).
  - proj + SwiGLU MLP: token-sharded (512 tokens/core), fully local.
v2 highlights vs the original baseline:
  - RoPE half-swap via SBUF->SBUF DMA instead of duplicate GEMM sections.
  - exp() batched as [128,1024] PSUM reads (amortizes ACT pipeline fill).
  - reciprocal_approx_fast for softmax denominators (was 3.4us each).
  - scores emitted one group ahead of attn@V to keep PE from head-of-line
    blocking on the exp semaphore.
  - v transposed as [128,128] full-tile PE transposes woven into the
    attention stream; Act.Silu fusion in the MLP.
"""

import functools
import numpy as np
import ml_dtypes

B, T, C, H, D = 2, 2048, 1024, 16, 64
HID = 4 * C
NCORES = 8
HPC = H // NCORES          # heads per core


def _build_program(b, t):
    import concourse.bacc as bacc
    import concourse.mybir as mybir
    import concourse.tile as tile
    import concourse.masks as masks
    from contextlib import ExitStack

    fp32 = mybir.dt.float32
    bf16 = mybir.dt.bfloat16
    Act = mybir.ActivationFunctionType
    Alu = mybir.AluOpType

    tok = b * t                    # all tokens (b-major)
    tpc = tok // NCORES            # tokens per core for proj/MLP/out
    half = tpc // 2                # tokens per (core, batch)
    kt_tiles = t // 128            # 128-token key tiles per batch
    tchunks = tok // 128           # transpose chunks over all tokens
    qt_chunk = min(512, t)
    qt_chunks = t // qt_chunk
    n_chunk = tok // 8             # QKV token chunks (8 PSUM banks)
    ck = C // 128                  # C chunks (8)
    mh_tiles = HID // 128          # hidden chunks (32)
    hg = 8                         # hidden chunks per weight-stream group
    gk = 2                         # key tiles per exp group
    g_cnt = kt_tiles // gk
    scale = float(D) ** -0.5

    nc = bacc.Bacc("TRN2", target_bir_lowering=False, debug=False,
                   num_devices=NCORES)

    # ---- DRAM I/O ----
    xT_d = nc.dram_tensor("xT", [C, tok], bf16, kind="ExternalInput")
    wqkvT_d = nc.dram_tensor("wqkvT", [C, 3 * 128], bf16, kind="ExternalInput")
    bqkv_d = nc.dram_tensor("bqkv2d", [128, 3], fp32, kind="ExternalInput")
    cos_d = nc.dram_tensor("cosd", [128, tok], bf16, kind="ExternalInput")
    sin_d = nc.dram_tensor("sind", [128, tok], bf16, kind="ExternalInput")
    wprojT_d = nc.dram_tensor("wprojT", [C, C], bf16, kind="ExternalInput")
    bproj_d = nc.dram_tensor("bproj2d", [128, ck], fp32, kind="ExternalInput")
    w1T_d = nc.dram_tensor("w1T", [C, HID], bf16, kind="ExternalInput")
    w2T_d = nc.dram_tensor("w2T", [C, HID], bf16, kind="ExternalInput")
    w3T_d = nc.dram_tensor("w3T", [HID, C], bf16, kind="ExternalInput")
    b1_d = nc.dram_tensor("b1_2d", [128, mh_tiles], fp32, kind="ExternalInput")
    b2_d = nc.dram_tensor("b2_2d", [128, mh_tiles], fp32, kind="ExternalInput")
    b3_d = nc.dram_tensor("b3_2d", [128, ck], fp32, kind="ExternalInput")
    y_d = nc.dram_tensor("y_loc", [C, tpc], fp32, kind="ExternalOutput")

    with tile.TileContext(nc) as tc:
        es = ExitStack()
        # ---- constants / biases (live whole kernel) ----
        consts = es.enter_context(tc.tile_pool(name="consts", bufs=1))
        ident = consts.tile([128, 128], bf16, name="ident")
        masks.make_identity(nc, ident[:])
        ones1 = consts.tile([65, 64], bf16, name="ones1")
        nc.vector.memset(ones1[:], 1.0)
        bqkv_sb = consts.tile([128, 3], fp32, name="bqkv_sb")
        nc.sync.dma_start(out=bqkv_sb[:], in_=bqkv_d[:, :])
        bproj_sb = consts.tile([128, ck], fp32, name="bproj_sb")
        nc.sync.dma_start(out=bproj_sb[:], in_=bproj_d[:, :])
        b1_sb = consts.tile([128, mh_tiles], fp32, name="b1_sb")
        nc.sync.dma_start(out=b1_sb[:], in_=b1_d[:, :])
        b2_sb = consts.tile([128, mh_tiles], fp32, name="b2_sb")
        nc.sync.dma_start(out=b2_sb[:], in_=b2_d[:, :])
        b3_sb = consts.tile([128, ck], fp32, name="b3_sb")
        nc.sync.dma_start(out=b3_sb[:], in_=b3_d[:, :])

        # ---- attention-lifetime tensors ----
        glob = es.enter_context(tc.tile_pool(name="glob", bufs=1))
        kr = glob.tile([128, tok], bf16, name="kr")
        qr = glob.tile([128, tok], bf16, name="qr")
        v_bf = glob.tile([128, tok], bf16, name="v_bf")
        v_aug = glob.tile([128, tchunks * 130], bf16, name="v_aug")
        nc.vector.memset(v_aug[:], 1.0)
        out_all = glob.tile([128, tok], bf16, name="out_all")

        # ---- DRAM bounce buffers for the two half-AllToAlls ----
        dram = es.enter_context(tc.tile_pool(name="dramp", bufs=1,
                                             space="DRAM"))
        a2a_in = [dram.tile([NCORES * 128, half], bf16, name=f"a2a_in{i}")
                  for i in range(2)]
        a2a_out = [dram.tile([NCORES * 128, half], bf16, name=f"a2a_out{i}")
                   for i in range(2)]

        # ================= Phase A: QKV GEMM + RoPE =================
        ph_a = ExitStack()
        xt_pool = ph_a.enter_context(tc.tile_pool(name="xt", bufs=1))
        wq_pool = ph_a.enter_context(tc.tile_pool(name="wq", bufs=1))
        rope_tab = ph_a.enter_context(tc.tile_pool(name="ropetab", bufs=1))
        sw_pool = ph_a.enter_context(tc.tile_pool(name="swp", bufs=1))
        qk_pool = ph_a.enter_context(tc.tile_pool(name="qkp", bufs=1))
        ps_qkv = ph_a.enter_context(
            tc.tile_pool(name="ps_qkv", bufs=1, space="PSUM"))

        wq_sb = []
        for kc in range(ck):
            wq_kc = wq_pool.tile([128, 3 * 128], bf16, name=f"wqkv{kc}")
            nc.sync.dma_start(out=wq_kc[:],
                              in_=wqkvT_d[128 * kc:128 * kc + 128, :])
            wq_sb.append(wq_kc)
        cos_sb = rope_tab.tile([128, tok], bf16, name="cos_sb")
        nc.sync.dma_start(out=cos_sb[:], in_=cos_d[:, :])
        sin_sb = rope_tab.tile([128, tok], bf16, name="sin_sb")
        nc.sync.dma_start(out=sin_sb[:], in_=sin_d[:, :])
        xt_sb = []
        for kc in range(ck):
            xt_kc = xt_pool.tile([128, tok], bf16, name=f"xt{kc}")
            nc.sync.dma_start(out=xt_kc[:], in_=xT_d[128 * kc:128 * kc + 128, :])
            xt_sb.append(xt_kc)

        k_bf = qk_pool.tile([128, tok], bf16, name="k_bf")
        q_bf = qk_pool.tile([128, tok], bf16, name="q_bf")
        ksw = sw_pool.tile([128, tok], bf16, name="ksw")
        qsw = sw_pool.tile([128, tok], bf16, name="qsw")

        ps_n = [ps_qkv.tile([128, n_chunk], fp32, name=f"psqkv{n}")
                for n in range(8)]
        dest = [k_bf, q_bf, v_bf]          # section order: k, q, v
        tcur = [0]                         # v-transpose chunk cursor
        for mi in range(3):
            for kc in range(ck):
                wslice = wq_sb[kc][:, 128 * mi:128 * mi + 128]
                for n in range(8):
                    nc.tensor.matmul(
                        ps_n[n][:], wslice,
                        xt_sb[kc][:, n_chunk * n:n_chunk * (n + 1)],
                        start=(kc == 0), stop=(kc == ck - 1))
            for n in range(8):
                nc.scalar.activation(
                    dest[mi][:, n_chunk * n:n_chunk * (n + 1)], ps_n[n][:],
                    Act.Identity, bias=bqkv_sb[:, mi:mi + 1])

            if mi <= 1:
                # RoPE for this section: half-swap via DMA, then
                # r = u*cos + u_sw*sin_signed.
                u_bf, u_sw, u_r = dest[mi], (ksw, qsw)[mi], (kr, qr)[mi]
                for (so, do) in ((0, 32), (32, 0), (64, 96), (96, 64)):
                    nc.sync.dma_start(out=u_sw[do:do + 32, :],
                                      in_=u_bf[so:so + 32, :])
                nc.vector.tensor_mul(u_r[:], u_bf[:], cos_sb[:])
                nc.vector.tensor_mul(u_bf[:], u_sw[:], sin_sb[:])
                nc.vector.tensor_add(u_r[:], u_r[:], u_bf[:])
        ph_a.close()

        # ---- long-lived compute tensors (opened after phase A frees SBUF) ----
        es2 = ExitStack()
        wp_pool = es2.enter_context(tc.tile_pool(name="wpp", bufs=1))
        w1g_pool = es2.enter_context(tc.tile_pool(name="w1g", bufs=2))
        w2g_pool = es2.enter_context(tc.tile_pool(name="w2g", bufs=2))
        w3_pool = es2.enter_context(tc.tile_pool(name="w3p", bufs=3))
        mlp_pool = es2.enter_context(tc.tile_pool(name="mlp", bufs=1))
        of_pool = es2.enter_context(tc.tile_pool(name="ofp", bufs=1))
        aT = mlp_pool.tile([128, ck * tpc], bf16, name="aT")

        # prefetch proj weights + first MLP weight groups during attention,
        # on the ACT-engine DMA ring (parallel to the SP ring's traffic)
        wp_sb = []
        for kc in range(ck):
            wp_kc = wp_pool.tile([128, C], bf16, name=f"wp{kc}")
            nc.scalar.dma_start(out=wp_kc[:],
                                in_=wprojT_d[128 * kc:128 * kc + 128, :])
            wp_sb.append(wp_kc)

        w1g_tiles = {}
        w2g_tiles = {}

        def load_mlp_group(g, eng):
            w1g = []
            w2g = []
            for kc in range(ck):
                w1k = w1g_pool.tile([128, hg * 128], bf16,
                                    name=f"w1g{g}_{kc}", tag=f"w1g{kc}")
                eng.dma_start(
                    out=w1k[:],
                    in_=w1T_d[128 * kc:128 * kc + 128,
                              hg * 128 * g:hg * 128 * (g + 1)])
                w1g.append(w1k)
                w2k = w2g_pool.tile([128, hg * 128], bf16,
                                    name=f"w2g{g}_{kc}", tag=f"w2g{kc}")
                eng.dma_start(
                    out=w2k[:],
                    in_=w2T_d[128 * kc:128 * kc + 128,
                              hg * 128 * g:hg * 128 * (g + 1)])
                w2g.append(w2k)
            w1g_tiles[g] = w1g
            w2g_tiles[g] = w2g

        load_mlp_group(0, nc.scalar)
        if mh_tiles // hg > 1:
            load_mlp_group(1, nc.scalar)

        # ---- v transpose into v_aug (PE, dense burst in the RoPE window) --
        ph_tr = ExitStack()
        ps_tr = ph_tr.enter_context(
            tc.tile_pool(name="ps_tr", bufs=4, space="PSUM"))
        for ci in range(tchunks):
            pst = ps_tr.tile([128, 128], bf16, name=f"pst{ci}", tag="pst")
            nc.tensor.transpose(pst[:], v_bf[:, 128 * ci:128 * ci + 128],
                                ident[:])
            base = ci * 130
            if ci % 2 == 0:
                nc.vector.tensor_copy(v_aug[:, base:base + 64], pst[:, 0:64])
                nc.vector.tensor_copy(v_aug[:, base + 65:base + 129],
                                      pst[:, 64:128])
            else:
                nc.scalar.activation(v_aug[:, base:base + 64], pst[:, 0:64],
                                     Act.Copy)
                nc.scalar.activation(v_aug[:, base + 65:base + 129],
                                     pst[:, 64:128], Act.Copy)
        ph_tr.close()

        # ================= Phase C: attention =================
        ph_c = ExitStack()
        ps_g = ph_c.enter_context(tc.tile_pool(name="ps_g", bufs=2,
                                               space="PSUM"))
        ps_o = ph_c.enter_context(tc.tile_pool(name="ps_o", bufs=3,
                                               space="PSUM"))
        ps_b = ph_c.enter_context(tc.tile_pool(name="ps_b", bufs=1,
                                               space="PSUM"))
        exp_pool = ph_c.enter_context(tc.tile_pool(name="expp", bufs=2))
        sm_pool = ph_c.enter_context(tc.tile_pool(name="smp", bufs=2))

        of_sb = [[None] * ck, [None] * ck]

        def emit_stage_cc(bi):
            for j in range(NCORES):
                c0 = bi * t + half * j
                nc.sync.dma_start(
                    out=a2a_in[bi][128 * j:128 * j + 128, :],
                    in_=out_all[:, c0:c0 + half])
            nc.gpsimd.collective_compute(
                "AllToAll", Alu.bypass,
                replica_groups=[list(range(NCORES))],
                ins=[a2a_in[bi][:]], outs=[a2a_out[bi][:]])

        def emit_of_load(bi):
            for kc in range(ck):
                of_kc = of_pool.tile([128, half], bf16, name=f"of{bi}_{kc}")
                nc.sync.dma_start(out=of_kc[:],
                                  in_=a2a_out[bi][128 * kc:128 * kc + 128, :])
                of_sb[bi][kc] = of_kc

        def emit_proj_m(bi, m, pool):
            psa = pool.tile([128, half], fp32, name=f"psa{bi}_{m}", tag="psa")
            for kc in range(ck):
                nc.tensor.matmul(psa[:], wp_sb[kc][:, 128 * m:128 * m + 128],
                                 of_sb[bi][kc][:],
                                 start=(kc == 0), stop=(kc == ck - 1))
            nc.vector.tensor_scalar(
                out=aT[:, tpc * m + half * bi:tpc * m + half * (bi + 1)],
                in0=psa[:], scalar1=bproj_sb[:, m:m + 1], scalar2=None,
                op0=Alu.add)

        # one pass per (batch, query chunk); both heads interleaved per
        # group so PE stays dense (keeps the HAM clock warm) and the exp
        # stream on ScalarE is the only pacer.
        passes = [(bi, qc) for bi in range(b) for qc in range(qt_chunks)]
        for pi, (bi, qc) in enumerate(passes):
            q0 = bi * t + qc * qt_chunk
            horder = (0, 1) if pi % 2 == 0 else (1, 0)
            expTs = {}
            psos = {}
            for h in horder:
                expTs[h] = exp_pool.tile([128, kt_tiles * qt_chunk], bf16,
                                         name=f"expT{pi}_{h}", tag="expT")
                psos[h] = ps_o.tile([65, qt_chunk], fp32,
                                    name=f"pso{pi}_{h}", tag="pso")

            def emit_attnv(g):
                for h in horder:
                    for j in range(gk):
                        kt = gk * g + j
                        ci = bi * kt_tiles + kt
                        vblk = v_aug[:, ci * 130 + 65 * h:
                                     ci * 130 + 65 * h + 65]
                        nc.tensor.matmul(
                            psos[h][:], vblk,
                            expTs[h][:, qt_chunk * kt:qt_chunk * (kt + 1)],
                            start=(kt == 0), stop=(kt == kt_tiles - 1))

            for g in range(g_cnt):
                gps = {}
                for h in horder:
                    gp = ps_g.tile([128, gk * qt_chunk], fp32,
                                   name=f"gps{pi}_{g}_{h}", tag="gps")
                    for j in range(gk):
                        kt = gk * g + j
                        nc.tensor.matmul(
                            gp[:, qt_chunk * j:qt_chunk * (j + 1)],
                            kr[64 * h:64 * h + 64,
                               bi * t + 128 * kt:bi * t + 128 * kt + 128],
                            qr[64 * h:64 * h + 64, q0:q0 + qt_chunk],
                            start=True, stop=True)
                    gps[h] = gp
                for h in horder:
                    nc.scalar.activation(
                        expTs[h][:, gk * qt_chunk * g:gk * qt_chunk * (g + 1)],
                        gps[h][:], Act.Exp, scale=scale)
                if g >= 1:
                    emit_attnv(g - 1)
            emit_attnv(g_cnt - 1)

            # softmax normalization (denominators live in pso row 64)
            for h in horder:
                pso = psos[h]
                recip = sm_pool.tile([65, qt_chunk], fp32,
                                     name=f"rc{pi}_{h}", tag="rc")
                nc.vector.reciprocal(recip[64:65, :], pso[64:65, :])
                recipb = sm_pool.tile([65, qt_chunk], bf16,
                                      name=f"rb{pi}_{h}", tag="rb")
                nc.vector.tensor_copy(recipb[64:65, :], recip[64:65, :])
                psb = ps_b.tile([64, qt_chunk], fp32, name=f"psb{pi}_{h}",
                                tag="psb")
                nc.tensor.matmul(psb[:], ones1[64:65, :], recipb[64:65, :],
                                 start=True, stop=True)
                bcast = sm_pool.tile([64, qt_chunk], fp32,
                                     name=f"bc{pi}_{h}", tag="bc")
                nc.vector.tensor_copy(bcast[:], psb[:])
                nc.vector.tensor_mul(
                    out_all[64 * h:64 * h + 64, q0:q0 + qt_chunk],
                    pso[0:64, :], bcast[:])

            if pi == qt_chunks - 1:
                emit_stage_cc(0)
                emit_of_load(0)

        emit_stage_cc(1)
        ph_c.close()

        # hT allocated after attention scratch is freed (reuses expT space)
        es3 = ExitStack()
        ht_pool = es3.enter_context(tc.tile_pool(name="htp", bufs=1))
        hT = ht_pool.tile([128, mh_tiles * tpc], bf16, name="hT")

        # ============ Phase E: proj (b0 overlaps the b1 collective) =====
        ph_e = ExitStack()
        ps_a2 = ph_e.enter_context(tc.tile_pool(name="ps_a2", bufs=2,
                                                space="PSUM"))
        for m in range(ck):
            emit_proj_m(0, m, ps_a2)
        emit_of_load(1)
        for m in range(ck):
            emit_proj_m(1, m, ps_a2)
        ph_e.close()

        # ================= Phase F: SwiGLU MLP =================
        ph_f = ExitStack()
        ps_1 = ph_f.enter_context(tc.tile_pool(name="ps_1", bufs=2,
                                               space="PSUM"))
        ps_2 = ph_f.enter_context(tc.tile_pool(name="ps_2", bufs=2,
                                               space="PSUM"))
        s_pool = ph_f.enter_context(tc.tile_pool(name="sp", bufs=2))

        for g in range(mh_tiles // hg):
            if g not in w1g_tiles:
                load_mlp_group(g, nc.sync)
            if g + 2 < mh_tiles // hg and g + 2 not in w1g_tiles:
                load_mlp_group(g + 2, nc.sync)
            w1g = w1g_tiles[g]
            w2g = w2g_tiles[g]
            for ml in range(hg):
                mh = hg * g + ml
                ps1 = ps_1.tile([128, tpc], fp32, name=f"ps1_{mh}", tag="ps1")
                ps2 = ps_2.tile([128, tpc], fp32, name=f"ps2_{mh}", tag="ps2")
                for kc in range(ck):
                    nc.tensor.matmul(ps1[:],
                                     w1g[kc][:, 128 * ml:128 * ml + 128],
                                     aT[:, tpc * kc:tpc * (kc + 1)],
                                     start=(kc == 0), stop=(kc == ck - 1))
                for kc in range(ck):
                    nc.tensor.matmul(ps2[:],
                                     w2g[kc][:, 128 * ml:128 * ml + 128],
                                     aT[:, tpc * kc:tpc * (kc + 1)],
                                     start=(kc == 0), stop=(kc == ck - 1))
                g_sb = s_pool.tile([128, tpc], fp32, name=f"g{mh}", tag="g")
                nc.scalar.activation(g_sb[:], ps1[:], Act.Sigmoid,
                                     bias=b1_sb[:, mh:mh + 1])
                s_sb = s_pool.tile([128, tpc], fp32, name=f"s{mh}", tag="s")
                nc.vector.scalar_tensor_tensor(
                    s_sb[:], ps1[:], b1_sb[:, mh:mh + 1], g_sb[:],
                    op0=Alu.add, op1=Alu.mult)
                nc.vector.scalar_tensor_tensor(
                    hT[:, tpc * mh:tpc * (mh + 1)], ps2[:],
                    b2_sb[:, mh:mh + 1], s_sb[:],
                    op0=Alu.add, op1=Alu.mult)
        ph_f.close()

        # ================= Phase G: w3 =================
        ph_g = ExitStack()
        ps_3 = ph_g.enter_context(tc.tile_pool(name="ps_3", bufs=1,
                                               space="PSUM"))
        out_pool = ph_g.enter_context(tc.tile_pool(name="outp", bufs=2))
        ps_3m = [ps_3.tile([128, tpc], fp32, name=f"ps3_{m}")
                 for m in range(ck)]
        for kh in range(mh_tiles):
            w3k = w3_pool.tile([128, C], bf16, name=f"w3k{kh}", tag="w3k")
            nc.sync.dma_start(out=w3k[:],
                              in_=w3T_d[128 * kh:128 * kh + 128, :])
            for m in range(ck):
                nc.tensor.matmul(ps_3m[m][:],
                                 w3k[:, 128 * m:128 * m + 128],
                                 hT[:, tpc * kh:tpc * (kh + 1)],
                                 start=(kh == 0), stop=(kh == mh_tiles - 1))
        for m in range(ck):
            yo = out_pool.tile([128, tpc], fp32, name=f"yo{m}", tag="yo")
            nc.scalar.activation(yo[:], ps_3m[m][:], Act.Identity,
                                 bias=b3_sb[:, m:m + 1])
            nc.sync.dma_start(out=y_d[128 * m:128 * m + 128, :], in_=yo[:])
        ph_g.close()
        es3.close()
        es2.close()
        es.close()

    nc.compile()
    return nc


@functools.lru_cache(maxsize=2)
def _get_program(b, t):
    return _build_program(b, t)


def _prep_inputs(x, w_qkv, b_qkv, w_proj, b_proj, w1, b1, w2, b2, w3, b3,
                 cos, sin, b, t):
    """Build per-core in_maps (host-side sharding / transposes / casts)."""
    bf = ml_dtypes.bfloat16
    tok = b * t
    ck = C // 128
    mh_tiles = HID // 128

    xT = np.ascontiguousarray(x.reshape(tok, C).T).astype(bf)
    # RoPE tables tiled to [128, tok]: rows = 4x the 32 freq rows,
    # cols = b-major tokens.
    cosd = np.tile(cos.T, (4, b)).astype(bf)
    sind = np.tile(np.concatenate([-sin.T, sin.T], axis=0), (2, b)).astype(bf)
    wprojT = np.ascontiguousarray(w_proj.T).astype(bf)
    bproj2d = np.ascontiguousarray(b_proj.reshape(ck, 128).T).astype(np.float32)
    w1T = np.ascontiguousarray(w1.T).astype(bf)
    w2T = np.ascontiguousarray(w2.T).astype(bf)
    w3T = np.ascontiguousarray(w3.T).astype(bf)
    b1_2d = np.ascontiguousarray(b1.reshape(mh_tiles, 128).T).astype(np.float32)
    b2_2d = np.ascontiguousarray(b2.reshape(mh_tiles, 128).T).astype(np.float32)
    b3_2d = np.ascontiguousarray(b3.reshape(ck, 128).T).astype(np.float32)

    # even/odd RoPE permutation within each head's 64 dims
    perm = np.concatenate([np.arange(0, D, 2), np.arange(1, D, 2)])

    in_maps = []
    for c in range(NCORES):
        rows = []
        brows = []
        secperm = [(1, perm), (0, perm), (2, np.arange(D))]   # k, q, v
        for sec, p in secperm:
            for hh in range(HPC):
                h = HPC * c + hh
                idx = sec * H * D + h * D + p
                rows.append(w_qkv[idx, :])
                brows.append(b_qkv[idx])
        wql = np.concatenate(rows, axis=0)           # [384, C]
        bql = np.concatenate(brows, axis=0)          # [384]
        wqkvT = np.ascontiguousarray(wql.T).astype(bf)
        bqkv2d = np.ascontiguousarray(bql.reshape(3, 128).T).astype(np.float32)
        in_maps.append({
            "xT": xT, "wqkvT": wqkvT, "bqkv2d": bqkv2d,
            "cosd": cosd, "sind": sind,
            "wprojT": wprojT, "bproj2d": bproj2d,
            "w1T": w1T, "w2T": w2T, "w3T": w3T,
            "b1_2d": b1_2d, "b2_2d": b2_2d, "b3_2d": b3_2d,
        })
    return in_maps


def kernel(x, w_qkv, b_qkv, w_proj, b_proj, w1, b1, w2, b2, w3, b3, cos, sin,
           _trace=False):
    from concourse import bass_utils

    b, t, c = x.shape
    assert (b, t, c) == (B, T, C)
    args = [np.asarray(a, dtype=np.float32) for a in
            (x, w_qkv, b_qkv, w_proj, b_proj, w1, b1, w2, b2, w3, b3,
             cos, sin)]
    nc = _get_program(b, t)
    in_maps = _prep_inputs(*args, b, t)
    res = bass_utils.run_bass_kernel_spmd(
        nc, in_maps, core_ids=list(range(NCORES)), trace=_trace)
    tpc = (b * t) // NCORES
    half = tpc // 2
    y = np.empty((b * t, c), dtype=np.float32)
    for i in range(NCORES):
        yl = res.results[i]["y_loc"]
        for bi in range(b):
            y[bi * t + half * i: bi * t + half * (i + 1), :] = \
                yl[:, half * bi:half * (bi + 1)].T
    out = y.reshape(b, t, c)
    if _trace:
        return out, res
    return out


# revision 34
# speedup vs baseline: 1.0531x; 1.0531x over previous
"""Trainium2 Bass kernel for a dense transformer block (RoPE attention + SwiGLU).

Sharding (8 NeuronCores, Megatron-style):
  - QKV + attention: tensor-parallel over heads (2 heads/core, both batches).
  - Two half-AllToAlls (one per batch) reshard attention output from
    head-sharded to token-sharded; the batch-0 collective overlaps with
    batch-1 attention compute (token slices are interleaved per batch so
    both collectives are symmetric).
  - proj + SwiGLU MLP: token-sharded (512 tokens/core), fully local.
v2 highlights vs the original baseline:
  - RoPE half-swap via SBUF->SBUF DMA instead of duplicate GEMM sections.
  - exp() batched as [128,1024] PSUM reads (amortizes ACT pipeline fill).
  - reciprocal_approx_fast for softmax denominators (was 3.4us each).
  - scores emitted one group ahead of attn@V to keep PE from head-of-line
    blocking on the exp semaphore.
  - v transposed as [128,128] full-tile PE transposes woven into the
    attention stream; Act.Silu fusion in the MLP.
"""

import functools
import numpy as np
import ml_dtypes

B, T, C, H, D = 2, 2048, 1024, 16, 64
HID = 4 * C
NCORES = 8
HPC = H // NCORES          # heads per core


def _build_program(b, t):
    import concourse.bacc as bacc
    import concourse.mybir as mybir
    import concourse.tile as tile
    import concourse.masks as masks
    from contextlib import ExitStack

    fp32 = mybir.dt.float32
    bf16 = mybir.dt.bfloat16
    Act = mybir.ActivationFunctionType
    Alu = mybir.AluOpType

    tok = b * t                    # all tokens (b-major)
    tpc = tok // NCORES            # tokens per core for proj/MLP/out
    half = tpc // 2                # tokens per (core, batch)
    kt_tiles = t // 128            # 128-token key tiles per batch
    tchunks = tok // 128           # transpose chunks over all tokens
    qt_chunk = min(512, t)
    qt_chunks = t // qt_chunk
    n_chunk = tok // 8             # QKV token chunks (8 PSUM banks)
    ck = C // 128                  # C chunks (8)
    mh_tiles = HID // 128          # hidden chunks (32)
    hg = 8                         # hidden chunks per weight-stream group
    gk = 2                         # key tiles per exp group
    g_cnt = kt_tiles // gk
    scale = float(D) ** -0.5

    nc = bacc.Bacc("TRN2", target_bir_lowering=False, debug=False,
                   num_devices=NCORES)

    # ---- DRAM I/O ----
    xT_d = nc.dram_tensor("xT", [C, tok], bf16, kind="ExternalInput")
    wqkvT_d = nc.dram_tensor("wqkvT", [C, 3 * 128], bf16, kind="ExternalInput")
    bqkv_d = nc.dram_tensor("bqkv2d", [128, 3], fp32, kind="ExternalInput")
    cos_d = nc.dram_tensor("cosd", [128, tok], bf16, kind="ExternalInput")
    sin_d = nc.dram_tensor("sind", [128, tok], bf16, kind="ExternalInput")
    wprojT_d = nc.dram_tensor("wprojT", [C, C], bf16, kind="ExternalInput")
    bproj_d = nc.dram_tensor("bproj2d", [128, ck], fp32, kind="ExternalInput")
    w1T_d = nc.dram_tensor("w1T", [C, HID], bf16, kind="ExternalInput")
    w2T_d = nc.dram_tensor("w2T", [C, HID], bf16, kind="ExternalInput")
    w3T_d = nc.dram_tensor("w3T", [HID, C], bf16, kind="ExternalInput")
    b1_d = nc.dram_tensor("b1_2d", [128, mh_tiles], fp32, kind="ExternalInput")
    b2_d = nc.dram_tensor("b2_2d", [128, mh_tiles], fp32, kind="ExternalInput")
    b3_d = nc.dram_tensor("b3_2d", [128, ck], fp32, kind="ExternalInput")
    y_d = nc.dram_tensor("y_loc", [C, tpc], fp32, kind="ExternalOutput")

    with tile.TileContext(nc) as tc:
        es = ExitStack()
        # ---- constants / biases (live whole kernel) ----
        consts = es.enter_context(tc.tile_pool(name="consts", bufs=1))
        ident = consts.tile([128, 128], bf16, name="ident")
        masks.make_identity(nc, ident[:])
        ones1 = consts.tile([65, 64], bf16, name="ones1")
        nc.vector.memset(ones1[:], 1.0)
        bqkv_sb = consts.tile([128, 3], fp32, name="bqkv_sb")
        nc.sync.dma_start(out=bqkv_sb[:], in_=bqkv_d[:, :])
        bproj_sb = consts.tile([128, ck], fp32, name="bproj_sb")
        nc.sync.dma_start(out=bproj_sb[:], in_=bproj_d[:, :])
        b1_sb = consts.tile([128, mh_tiles], fp32, name="b1_sb")
        nc.sync.dma_start(out=b1_sb[:], in_=b1_d[:, :])
        b2_sb = consts.tile([128, mh_tiles], fp32, name="b2_sb")
        nc.sync.dma_start(out=b2_sb[:], in_=b2_d[:, :])
        b3_sb = consts.tile([128, ck], fp32, name="b3_sb")
        nc.sync.dma_start(out=b3_sb[:], in_=b3_d[:, :])

        # ---- attention-lifetime tensors ----
        glob = es.enter_context(tc.tile_pool(name="glob", bufs=1))
        kr = glob.tile([128, tok], bf16, name="kr")
        qr = glob.tile([128, tok], bf16, name="qr")
        v_bf = glob.tile([128, tok], bf16, name="v_bf")
        v_aug = glob.tile([128, tchunks * 130], bf16, name="v_aug")
        nc.vector.memset(v_aug[:], 1.0)
        out_all = glob.tile([128, tok], bf16, name="out_all")

        # ---- DRAM bounce buffers for the two half-AllToAlls ----
        dram = es.enter_context(tc.tile_pool(name="dramp", bufs=1,
                                             space="DRAM"))
        a2a_in = [dram.tile([NCORES * 128, half], bf16, name=f"a2a_in{i}")
                  for i in range(2)]
        a2a_out = [dram.tile([NCORES * 128, half], bf16, name=f"a2a_out{i}")
                   for i in range(2)]

        # ================= Phase A: QKV GEMM + RoPE =================
        ph_a = ExitStack()
        xt_pool = ph_a.enter_context(tc.tile_pool(name="xt", bufs=1))
        wq_pool = ph_a.enter_context(tc.tile_pool(name="wq", bufs=1))
        rope_tab = ph_a.enter_context(tc.tile_pool(name="ropetab", bufs=1))
        sw_pool = ph_a.enter_context(tc.tile_pool(name="swp", bufs=1))
        qk_pool = ph_a.enter_context(tc.tile_pool(name="qkp", bufs=1))
        ps_qkv = ph_a.enter_context(
            tc.tile_pool(name="ps_qkv", bufs=1, space="PSUM"))

        wq_sb = []
        for kc in range(ck):
            wq_kc = wq_pool.tile([128, 3 * 128], bf16, name=f"wqkv{kc}")
            nc.sync.dma_start(out=wq_kc[:],
                              in_=wqkvT_d[128 * kc:128 * kc + 128, :])
            wq_sb.append(wq_kc)
        cos_sb = rope_tab.tile([128, tok], bf16, name="cos_sb")
        nc.sync.dma_start(out=cos_sb[:], in_=cos_d[:, :])
        sin_sb = rope_tab.tile([128, tok], bf16, name="sin_sb")
        nc.sync.dma_start(out=sin_sb[:], in_=sin_d[:, :])
        xt_sb = []
        for kc in range(ck):
            xt_kc = xt_pool.tile([128, tok], bf16, name=f"xt{kc}")
            nc.sync.dma_start(out=xt_kc[:], in_=xT_d[128 * kc:128 * kc + 128, :])
            xt_sb.append(xt_kc)

        k_bf = qk_pool.tile([128, tok], bf16, name="k_bf")
        q_bf = qk_pool.tile([128, tok], bf16, name="q_bf")
        ksw = sw_pool.tile([128, tok], bf16, name="ksw")
        qsw = sw_pool.tile([128, tok], bf16, name="qsw")

        ps_n = [ps_qkv.tile([128, n_chunk], fp32, name=f"psqkv{n}")
                for n in range(8)]
        dest = [k_bf, q_bf, v_bf]          # section order: k, q, v
        tcur = [0]                         # v-transpose chunk cursor
        for mi in range(3):
            for kc in range(ck):
                wslice = wq_sb[kc][:, 128 * mi:128 * mi + 128]
                for n in range(8):
                    nc.tensor.matmul(
                        ps_n[n][:], wslice,
                        xt_sb[kc][:, n_chunk * n:n_chunk * (n + 1)],
                        start=(kc == 0), stop=(kc == ck - 1))
            for n in range(8):
                nc.scalar.activation(
                    dest[mi][:, n_chunk * n:n_chunk * (n + 1)], ps_n[n][:],
                    Act.Identity, bias=bqkv_sb[:, mi:mi + 1])

            if mi <= 1:
                # RoPE for this section: half-swap via DMA, then
                # r = u*cos + u_sw*sin_signed.
                u_bf, u_sw, u_r = dest[mi], (ksw, qsw)[mi], (kr, qr)[mi]
                for (so, do) in ((0, 32), (32, 0), (64, 96), (96, 64)):
                    nc.sync.dma_start(out=u_sw[do:do + 32, :],
                                      in_=u_bf[so:so + 32, :])
                nc.vector.tensor_mul(u_r[:], u_bf[:], cos_sb[:])
                nc.vector.tensor_mul(u_bf[:], u_sw[:], sin_sb[:])
                nc.vector.tensor_add(u_r[:], u_r[:], u_bf[:])
        ph_a.close()

        # ---- long-lived compute tensors (opened after phase A frees SBUF) ----
        es2 = ExitStack()
        wp_pool = es2.enter_context(tc.tile_pool(name="wpp", bufs=1))
        w1g_pool = es2.enter_context(tc.tile_pool(name="w1g", bufs=2))
        w2g_pool = es2.enter_context(tc.tile_pool(name="w2g", bufs=2))
        w3_pool = es2.enter_context(tc.tile_pool(name="w3p", bufs=3))
        mlp_pool = es2.enter_context(tc.tile_pool(name="mlp", bufs=1))
        of_pool = es2.enter_context(tc.tile_pool(name="ofp", bufs=1))
        aT = mlp_pool.tile([128, ck * tpc], bf16, name="aT")

        # prefetch proj weights + first MLP weight groups during attention,
        # on the ACT-engine DMA ring (parallel to the SP ring's traffic)
        wp_sb = []
        for kc in range(ck):
            wp_kc = wp_pool.tile([128, C], bf16, name=f"wp{kc}")
            nc.sync.dma_start(out=wp_kc[:],
                              in_=wprojT_d[128 * kc:128 * kc + 128, :])
            wp_sb.append(wp_kc)

        w1g_tiles = {}
        w2g_tiles = {}

        def load_mlp_group(g, eng):
            w1g = []
            w2g = []
            for kc in range(ck):
                w1k = w1g_pool.tile([128, hg * 128], bf16,
                                    name=f"w1g{g}_{kc}", tag=f"w1g{kc}")
                eng.dma_start(
                    out=w1k[:],
                    in_=w1T_d[128 * kc:128 * kc + 128,
                              hg * 128 * g:hg * 128 * (g + 1)])
                w1g.append(w1k)
                w2k = w2g_pool.tile([128, hg * 128], bf16,
                                    name=f"w2g{g}_{kc}", tag=f"w2g{kc}")
                eng.dma_start(
                    out=w2k[:],
                    in_=w2T_d[128 * kc:128 * kc + 128,
                              hg * 128 * g:hg * 128 * (g + 1)])
                w2g.append(w2k)
            w1g_tiles[g] = w1g
            w2g_tiles[g] = w2g

        load_mlp_group(0, nc.sync)
        if mh_tiles // hg > 1:
            load_mlp_group(1, nc.sync)

        # ---- v transpose into v_aug (PE, dense burst in the RoPE window) --
        ph_tr = ExitStack()
        ps_tr = ph_tr.enter_context(
            tc.tile_pool(name="ps_tr", bufs=4, space="PSUM"))
        for ci in range(tchunks):
            pst = ps_tr.tile([128, 128], bf16, name=f"pst{ci}", tag="pst")
            nc.tensor.transpose(pst[:], v_bf[:, 128 * ci:128 * ci + 128],
                                ident[:])
            base = ci * 130
            if ci % 2 == 0:
                nc.vector.tensor_copy(v_aug[:, base:base + 64], pst[:, 0:64])
                nc.vector.tensor_copy(v_aug[:, base + 65:base + 129],
                                      pst[:, 64:128])
            else:
                nc.scalar.activation(v_aug[:, base:base + 64], pst[:, 0:64],
                                     Act.Copy)
                nc.scalar.activation(v_aug[:, base + 65:base + 129],
                                     pst[:, 64:128], Act.Copy)
        ph_tr.close()

        # ================= Phase C: attention =================
        ph_c = ExitStack()
        ps_g = ph_c.enter_context(tc.tile_pool(name="ps_g", bufs=2,
                                               space="PSUM"))
        ps_o = ph_c.enter_context(tc.tile_pool(name="ps_o", bufs=3,
                                               space="PSUM"))
        ps_b = ph_c.enter_context(tc.tile_pool(name="ps_b", bufs=1,
                                               space="PSUM"))
        exp_pool = ph_c.enter_context(tc.tile_pool(name="expp", bufs=2))
        sm_pool = ph_c.enter_context(tc.tile_pool(name="smp", bufs=2))

        of_sb = [[None] * ck, [None] * ck]

        def emit_stage_cc(bi):
            for j in range(NCORES):
                c0 = bi * t + half * j
                nc.sync.dma_start(
                    out=a2a_in[bi][128 * j:128 * j + 128, :],
                    in_=out_all[:, c0:c0 + half])
            nc.gpsimd.collective_compute(
                "AllToAll", Alu.bypass,
                replica_groups=[list(range(NCORES))],
                ins=[a2a_in[bi][:]], outs=[a2a_out[bi][:]])

        def emit_of_load(bi):
            for kc in range(ck):
                of_kc = of_pool.tile([128, half], bf16, name=f"of{bi}_{kc}")
                nc.sync.dma_start(out=of_kc[:],
                                  in_=a2a_out[bi][128 * kc:128 * kc + 128, :])
                of_sb[bi][kc] = of_kc

        def emit_proj_m(bi, m, pool):
            psa = pool.tile([128, half], fp32, name=f"psa{bi}_{m}", tag="psa")
            for kc in range(ck):
                nc.tensor.matmul(psa[:], wp_sb[kc][:, 128 * m:128 * m + 128],
                                 of_sb[bi][kc][:],
                                 start=(kc == 0), stop=(kc == ck - 1))
            nc.vector.tensor_scalar(
                out=aT[:, tpc * m + half * bi:tpc * m + half * (bi + 1)],
                in0=psa[:], scalar1=bproj_sb[:, m:m + 1], scalar2=None,
                op0=Alu.add)

        # one pass per (batch, query chunk); both heads interleaved per
        # group so PE stays dense (keeps the HAM clock warm) and the exp
        # stream on ScalarE is the only pacer.
        passes = [(bi, qc) for bi in range(b) for qc in range(qt_chunks)]
        for pi, (bi, qc) in enumerate(passes):
            q0 = bi * t + qc * qt_chunk
            horder = (0, 1) if pi % 2 == 0 else (1, 0)
            expTs = {}
            psos = {}
            for h in horder:
                expTs[h] = exp_pool.tile([128, kt_tiles * qt_chunk], bf16,
                                         name=f"expT{pi}_{h}", tag="expT")
                psos[h] = ps_o.tile([65, qt_chunk], fp32,
                                    name=f"pso{pi}_{h}", tag="pso")

            def emit_attnv(g):
                for h in horder:
                    for j in range(gk):
                        kt = gk * g + j
                        ci = bi * kt_tiles + kt
                        vblk = v_aug[:, ci * 130 + 65 * h:
                                     ci * 130 + 65 * h + 65]
                        nc.tensor.matmul(
                            psos[h][:], vblk,
                            expTs[h][:, qt_chunk * kt:qt_chunk * (kt + 1)],
                            start=(kt == 0), stop=(kt == kt_tiles - 1))

            for g in range(g_cnt):
                gps = {}
                for h in horder:
                    gps[h] = ps_g.tile([128, gk * qt_chunk], fp32,
                                       name=f"gps{pi}_{g}_{h}", tag="gps")
                # j-outer, h-inner: adjacent matmuls hit disjoint PE row
                # groups (h0 rows 0-63, h1 rows 64-127) and run concurrently
                for j in range(gk):
                    kt = gk * g + j
                    for h in horder:
                        nc.tensor.matmul(
                            gps[h][:, qt_chunk * j:qt_chunk * (j + 1)],
                            kr[64 * h:64 * h + 64,
                               bi * t + 128 * kt:bi * t + 128 * kt + 128],
                            qr[64 * h:64 * h + 64, q0:q0 + qt_chunk],
                            start=True, stop=True)
                for h in horder:
                    nc.scalar.activation(
                        expTs[h][:, gk * qt_chunk * g:gk * qt_chunk * (g + 1)],
                        gps[h][:], Act.Exp, scale=scale)
                if g >= 1:
                    emit_attnv(g - 1)
            emit_attnv(g_cnt - 1)

            # softmax normalization (denominators live in pso row 64)
            for h in horder:
                pso = psos[h]
                recip = sm_pool.tile([65, qt_chunk], fp32,
                                     name=f"rc{pi}_{h}", tag="rc")
                nc.vector.reciprocal(recip[64:65, :], pso[64:65, :])
                recipb = sm_pool.tile([65, qt_chunk], bf16,
                                      name=f"rb{pi}_{h}", tag="rb")
                nc.vector.tensor_copy(recipb[64:65, :], recip[64:65, :])
                psb = ps_b.tile([64, qt_chunk], fp32, name=f"psb{pi}_{h}",
                                tag="psb")
                nc.tensor.matmul(psb[:], ones1[64:65, :], recipb[64:65, :],
                                 start=True, stop=True)
                bcast = sm_pool.tile([64, qt_chunk], fp32,
                                     name=f"bc{pi}_{h}", tag="bc")
                nc.vector.tensor_copy(bcast[:], psb[:])
                nc.vector.tensor_mul(
                    out_all[64 * h:64 * h + 64, q0:q0 + qt_chunk],
                    pso[0:64, :], bcast[:])

            if pi == qt_chunks - 1:
                emit_stage_cc(0)
                emit_of_load(0)

        emit_stage_cc(1)
        ph_c.close()

        # hT allocated after attention scratch is freed (reuses expT space)
        es3 = ExitStack()
        ht_pool = es3.enter_context(tc.tile_pool(name="htp", bufs=1))
        hT = ht_pool.tile([128, mh_tiles * tpc], bf16, name="hT")

        # ============ Phase E: proj (b0 overlaps the b1 collective) =====
        ph_e = ExitStack()
        ps_a2 = ph_e.enter_context(tc.tile_pool(name="ps_a2", bufs=2,
                                                space="PSUM"))
        for m in range(ck):
            emit_proj_m(0, m, ps_a2)
        emit_of_load(1)
        for m in range(ck):
            emit_proj_m(1, m, ps_a2)
        ph_e.close()

        # ================= Phase F: SwiGLU MLP =================
        ph_f = ExitStack()
        ps_1 = ph_f.enter_context(tc.tile_pool(name="ps_1", bufs=2,
                                               space="PSUM"))
        ps_2 = ph_f.enter_context(tc.tile_pool(name="ps_2", bufs=2,
                                               space="PSUM"))
        s_pool = ph_f.enter_context(tc.tile_pool(name="sp", bufs=2))

        for g in range(mh_tiles // hg):
            if g not in w1g_tiles:
                load_mlp_group(g, nc.sync)
            if g + 2 < mh_tiles // hg and g + 2 not in w1g_tiles:
                load_mlp_group(g + 2, nc.sync)
            w1g = w1g_tiles[g]
            w2g = w2g_tiles[g]
            for ml in range(hg):
                mh = hg * g + ml
                ps1 = ps_1.tile([128, tpc], fp32, name=f"ps1_{mh}", tag="ps1")
                ps2 = ps_2.tile([128, tpc], fp32, name=f"ps2_{mh}", tag="ps2")
                for kc in range(ck):
                    nc.tensor.matmul(ps1[:],
                                     w1g[kc][:, 128 * ml:128 * ml + 128],
                                     aT[:, tpc * kc:tpc * (kc + 1)],
                                     start=(kc == 0), stop=(kc == ck - 1))
                for kc in range(ck):
                    nc.tensor.matmul(ps2[:],
                                     w2g[kc][:, 128 * ml:128 * ml + 128],
                                     aT[:, tpc * kc:tpc * (kc + 1)],
                                     start=(kc == 0), stop=(kc == ck - 1))
                g_sb = s_pool.tile([128, tpc], fp32, name=f"g{mh}", tag="g")
                nc.scalar.activation(g_sb[:], ps1[:], Act.Sigmoid,
                                     bias=b1_sb[:, mh:mh + 1])
                s_sb = s_pool.tile([128, tpc], fp32, name=f"s{mh}", tag="s")
                nc.vector.scalar_tensor_tensor(
                    s_sb[:], ps1[:], b1_sb[:, mh:mh + 1], g_sb[:],
                    op0=Alu.add, op1=Alu.mult)
                nc.vector.scalar_tensor_tensor(
                    hT[:, tpc * mh:tpc * (mh + 1)], ps2[:],
                    b2_sb[:, mh:mh + 1], s_sb[:],
                    op0=Alu.add, op1=Alu.mult)
        ph_f.close()

        # ================= Phase G: w3 =================
        ph_g = ExitStack()
        ps_3 = ph_g.enter_context(tc.tile_pool(name="ps_3", bufs=1,
                                               space="PSUM"))
        out_pool = ph_g.enter_context(tc.tile_pool(name="outp", bufs=2))
        ps_3m = [ps_3.tile([128, tpc], fp32, name=f"ps3_{m}")
                 for m in range(ck)]
        for kh in range(mh_tiles):
            w3k = w3_pool.tile([128, C], bf16, name=f"w3k{kh}", tag="w3k")
            nc.sync.dma_start(out=w3k[:],
                              in_=w3T_d[128 * kh:128 * kh + 128, :])
            for m in range(ck):
                nc.tensor.matmul(ps_3m[m][:],
                                 w3k[:, 128 * m:128 * m + 128],
                                 hT[:, tpc * kh:tpc * (kh + 1)],
                                 start=(kh == 0), stop=(kh == mh_tiles - 1))
        for m in range(ck):
            yo = out_pool.tile([128, tpc], fp32, name=f"yo{m}", tag="yo")
            nc.scalar.activation(yo[:], ps_3m[m][:], Act.Identity,
                                 bias=b3_sb[:, m:m + 1])
            nc.sync.dma_start(out=y_d[128 * m:128 * m + 128, :], in_=yo[:])
        ph_g.close()
        es3.close()
        es2.close()
        es.close()

    nc.compile()
    return nc


@functools.lru_cache(maxsize=2)
def _get_program(b, t):
    return _build_program(b, t)


def _prep_inputs(x, w_qkv, b_qkv, w_proj, b_proj, w1, b1, w2, b2, w3, b3,
                 cos, sin, b, t):
    """Build per-core in_maps (host-side sharding / transposes / casts)."""
    bf = ml_dtypes.bfloat16
    tok = b * t
    ck = C // 128
    mh_tiles = HID // 128

    xT = np.ascontiguousarray(x.reshape(tok, C).T).astype(bf)
    # RoPE tables tiled to [128, tok]: rows = 4x the 32 freq rows,
    # cols = b-major tokens.
    cosd = np.tile(cos.T, (4, b)).astype(bf)
    sind = np.tile(np.concatenate([-sin.T, sin.T], axis=0), (2, b)).astype(bf)
    wprojT = np.ascontiguousarray(w_proj.T).astype(bf)
    bproj2d = np.ascontiguousarray(b_proj.reshape(ck, 128).T).astype(np.float32)
    w1T = np.ascontiguousarray(w1.T).astype(bf)
    w2T = np.ascontiguousarray(w2.T).astype(bf)
    w3T = np.ascontiguousarray(w3.T).astype(bf)
    b1_2d = np.ascontiguousarray(b1.reshape(mh_tiles, 128).T).astype(np.float32)
    b2_2d = np.ascontiguousarray(b2.reshape(mh_tiles, 128).T).astype(np.float32)
    b3_2d = np.ascontiguousarray(b3.reshape(ck, 128).T).astype(np.float32)

    # even/odd RoPE permutation within each head's 64 dims
    perm = np.concatenate([np.arange(0, D, 2), np.arange(1, D, 2)])

    in_maps = []
    for c in range(NCORES):
        rows = []
        brows = []
        secperm = [(1, perm), (0, perm), (2, np.arange(D))]   # k, q, v
        for sec, p in secperm:
            for hh in range(HPC):
                h = HPC * c + hh
                idx = sec * H * D + h * D + p
                rows.append(w_qkv[idx, :])
                brows.append(b_qkv[idx])
        wql = np.concatenate(rows, axis=0)           # [384, C]
        bql = np.concatenate(brows, axis=0)          # [384]
        wqkvT = np.ascontiguousarray(wql.T).astype(bf)
        bqkv2d = np.ascontiguousarray(bql.reshape(3, 128).T).astype(np.float32)
        in_maps.append({
            "xT": xT, "wqkvT": wqkvT, "bqkv2d": bqkv2d,
            "cosd": cosd, "sind": sind,
            "wprojT": wprojT, "bproj2d": bproj2d,
            "w1T": w1T, "w2T": w2T, "w3T": w3T,
            "b1_2d": b1_2d, "b2_2d": b2_2d, "b3_2d": b3_2d,
        })
    return in_maps


def kernel(x, w_qkv, b_qkv, w_proj, b_proj, w1, b1, w2, b2, w3, b3, cos, sin,
           _trace=False):
    from concourse import bass_utils

    b, t, c = x.shape
    assert (b, t, c) == (B, T, C)
    args = [np.asarray(a, dtype=np.float32) for a in
            (x, w_qkv, b_qkv, w_proj, b_proj, w1, b1, w2, b2, w3, b3,
             cos, sin)]
    nc = _get_program(b, t)
    in_maps = _prep_inputs(*args, b, t)
    res = bass_utils.run_bass_kernel_spmd(
        nc, in_maps, core_ids=list(range(NCORES)), trace=_trace)
    tpc = (b * t) // NCORES
    half = tpc // 2
    y = np.empty((b * t, c), dtype=np.float32)
    for i in range(NCORES):
        yl = res.results[i]["y_loc"]
        for bi in range(b):
            y[bi * t + half * i: bi * t + half * (i + 1), :] = \
                yl[:, half * bi:half * (bi + 1)].T
    out = y.reshape(b, t, c)
    if _trace:
        return out, res
    return out
